# revision 1
# baseline (speedup 1.0000x reference)
"""Trainium2 Bass kernel for ContractiveInvertibleGNN feed-forward.

Math (reference, with group_mask == I_32):
  out[b,i] = f_i( sum_j W_adj[j,i] * g_j(X[b,j]) )
where g_j: R -> R^32 and f_i: R^32 -> R are slices of two shared MLPs
(64->128->128->32 with a residual middle block, LeakyReLU 0.01):
  g: H1 = lrelu(X[b,j]*U_j + C1_j); H2 = H1 + lrelu(H1@W2g + b2g)
     X_emb = H2 @ W3g + b3g
  f: in = [X_aggr ; emb_i] -> Hf1 = lrelu(X_aggr@Wf1x + C2_i)
     Hf2 = Hf1 + lrelu(Hf1@Wf2 + bf2); out_i = Hf2 . V_i (+ bf3_i)
with per-node constants U_j = g_W1[j,:], C1_j = emb_j@g_W1[32:]+g_b1,
C2_i = emb_i@f_W1[32:]+f_b1 (+ (sum_j W_adj[j,i])*g_b3@f_W1[:32]),
V_i = f_W3[:,i].

Sharding: pure data-parallel over batch across 8 cores (2048 rows each).

On-chip layout (per core): node-major columns. g-phase runs per node j over
[128, 2048] tiles; X_emb assembled as Xe[(c,d), (j,t)] with c = batch
quarter stacked on partition groups; StreamTranspose -> Xt[(c,j),(t,d)];
block-diag(W_adj) matmul aggregates over j; StreamTranspose back ->
Xa[(c,d),(i,t)]; f-phase per node i with padded stationaries selecting
partition group c; final dot with V_i via a [128,4] stationary that also
routes batch quarter c to psum row c.
"""

import os
import sys

import numpy as np

for _p in ("/opt/trn_rl_repo", "/root/.axon_site/_ro/trn_rl_repo"):
    if os.path.isdir(_p) and _p not in sys.path:
        sys.path.insert(0, _p)

N = 32          # nodes
D = 32          # processed dim (== N, group_mask = I)
A = 128         # hidden width
B = 16384       # batch
NCORES = 8
BC = B // NCORES        # 2048 rows per core
CH = 512                # matmul free-dim chunk
NCH = BC // CH          # 4 chunks (partition-group stacking factor)
ALPHA = 0.01

_F32R = None  # set lazily to mybir.dt.float32r


def _build_program(zero_b2=True):
    from contextlib import ExitStack

    from concourse import bacc, bass, mybir, tile

    global _F32R
    _F32R = mybir.dt.float32r
    f32 = mybir.dt.float32
    LRELU = mybir.ActivationFunctionType.Lrelu
    ALU_MULT = mybir.AluOpType.mult
    ALU_ADD = mybir.AluOpType.add
    ALU_MAX = mybir.AluOpType.max

    nc = bacc.Bacc("TRN2", target_bir_lowering=False, debug=False)

    f32r = mybir.dt.float32r

    def din(name, shape, dt=None):
        return nc.dram_tensor(
            name, list(shape), dt or f32r, kind="ExternalInput"
        ).ap()

    xt_d = din("XT", (N, BC), f32)
    gw2_d = din("GW2", (A, A))
    fw2_d = din("FW2", (A, A))
    gw3p_d = din("GW3P", (A, NCH * A))     # col-block c: rows of g_W3 at M cols 32c..
    fw1p_d = din("FW1P", (A, NCH * A))     # row-block c: f_W1[:32] at K rows 32c..
    bd_d = din("BD", (A, A))               # kron(I4, W_adj)
    u_d = din("U", (A, N), f32)
    c1_d = din("C1", (A, N), f32)
    c2_d = din("C2", (A, N), f32)
    gb2_d = din("GB2", (A, 1), f32)
    fb2_d = din("FB2", (A, 1), f32)
    vp_d = din("VP", (A, (N + 1) * D))     # [:, (i+1)*D] = V_i[a]; else 0
    out_d = nc.dram_tensor("OUT", [N, BC], f32, kind="ExternalOutput").ap()

    with tile.TileContext(nc) as tc, ExitStack() as ctx:
        const = ctx.enter_context(tc.tile_pool(name="const", bufs=1))
        bigp = ctx.enter_context(tc.tile_pool(name="big", bufs=2))
        workp = ctx.enter_context(tc.tile_pool(name="work", bufs=6))
        outp = ctx.enter_context(tc.tile_pool(name="outs", bufs=2))
        lrp = ctx.enter_context(tc.tile_pool(name="lrp", bufs=3))
        ppA = ctx.enter_context(tc.tile_pool(name="ppA", bufs=2, space="PSUM"))
        ppB = ctx.enter_context(tc.tile_pool(name="ppB", bufs=2, space="PSUM"))
        ppR = ctx.enter_context(tc.tile_pool(name="ppR", bufs=2, space="PSUM"))

        def load_const(ap_dram, shape):
            t = const.tile(list(shape), ap_dram.dtype,
                           tag=f"c_{ap_dram.tensor.name}")
            nc.sync.dma_start(t[:, :], ap_dram)
            return t

        gw2_s = load_const(gw2_d, (A, A))
        fw2_s = load_const(fw2_d, (A, A))
        gw3p_s = load_const(gw3p_d, (A, NCH * A))
        fw1p_s = load_const(fw1p_d, (A, NCH * A))
        bd_s = load_const(bd_d, (A, A))
        u_s = load_const(u_d, (A, N))
        c1_s = load_const(c1_d, (A, N))
        c2_s = load_const(c2_d, (A, N))
        gb2_s = load_const(gb2_d, (A, 1))
        fb2_s = load_const(fb2_d, (A, 1))
        vp_s = load_const(vp_d, (A, (N + 1) * D))


        # Xe[(c,d), (j,t)] = X_emb[d, j, c*CH+t]
        xe = bigp.tile([A, N * CH], f32r, tag="big")

        # ---------------- g phase: one node j per iteration ----------------
        for j in range(N):
            xbc = workp.tile([A, BC], f32, tag="w")
            nc.sync.dma_start(
                xbc[:, :], xt_d[j : j + 1, :].partition_broadcast(A)
            )
            h1 = workp.tile([A, BC], f32r, tag="w")
            SPL = 3 * CH
            nc.scalar.activation(
                h1[:, :SPL], xbc[:, :SPL], LRELU,
                bias=c1_s[:, j : j + 1], scale=u_s[:, j : j + 1], alpha=ALPHA,
            )
            zt = lrp.tile([A, CH], f32, tag="z")
            mt = lrp.tile([A, CH], f32, tag="m")
            nc.vector.tensor_scalar(zt[:, :], xbc[:, SPL:],
                                    u_s[:, j : j + 1], c1_s[:, j : j + 1],
                                    ALU_MULT, ALU_ADD)
            nc.vector.tensor_scalar(mt[:, :], zt[:, :], ALPHA, None, ALU_MULT)
            nc.vector.tensor_tensor(h1[:, SPL:], zt[:, :], mt[:, :], ALU_MAX)
            t2 = workp.tile([A, BC], f32r, tag="w")
            pm3 = ppB.tile([A, CH], f32, tag="pB")
            for h in range(2):  # halves of 1024 cols
                pa = ppA.tile([A, 2 * CH], f32, tag="pA")
                for q in range(2):
                    sl = slice(h * 2 * CH + q * CH, h * 2 * CH + (q + 1) * CH)
                    nc.tensor.matmul(
                        pa[:, q * CH : (q + 1) * CH], gw2_s[:, :],
                        h1[:, sl], start=True, stop=True,
                    )
                nc.scalar.activation(
                    t2[:, h * 2 * CH : (h + 1) * 2 * CH], pa[:, :], LRELU,
                    bias=gb2_s[:, 0:1], alpha=ALPHA,
                )
            # X_emb = g_W3^T @ (H1 + lrelu(.)) via 8 accumulating matmuls,
            # chunk c routed to psum rows 32c by the padded stationary.
            for c in range(NCH):
                lt = gw3p_s[:, c * A : (c + 1) * A]
                sl = slice(c * CH, (c + 1) * CH)
                nc.tensor.matmul(pm3[:, :], lt, h1[:, sl],
                                 start=(c == 0), stop=False)
                nc.tensor.matmul(pm3[:, :], lt, t2[:, sl],
                                 start=False, stop=(c == NCH - 1))
            nc.vector.tensor_copy(xe[:, j * CH : (j + 1) * CH], pm3[:, :])

        # ---------------- aggregation ----------------
        # T1: Xe[(c,d),(j,t)] -> Xt[(c,j),(t,d)]
        xt3 = xe.bitcast(f32).rearrange(
            "p (j t) -> p j t", j=N).transpose([0, 2, 1])
        xtile = bigp.tile([A, CH * D], f32, tag="big")
        xto = xtile.rearrange("p (t d) -> p t d", d=D)
        TS = 8  # split into 8 ops for overlap
        tstep = CH // TS
        for s in range(TS):
            nc.vector.transpose(
                xto[:, s * tstep : (s + 1) * tstep, :],
                xt3[:, s * tstep : (s + 1) * tstep, :],
            )
        # DMA hop: rounded-bits copy into an f32r-typed tensor for the PE
        xtile_r = bigp.tile([A, CH * D], f32r, tag="big")
        for s in range(TS):
            sl = slice(s * (CH * D // TS), (s + 1) * (CH * D // TS))
            nc.sync.dma_start(xtile_r[:, sl], xtile.bitcast(f32r)[:, sl])
        # agg windows + T2-back: psum[(c,i),(t16,d)] -> Xa[(c,d),(i,t)]
        xa = bigp.tile([A, N * CH], f32, tag="big")
        xa3 = xa.rearrange("p (i t) -> p i t", i=N).transpose([0, 2, 1])
        WT = CH // D  # 16 t per window
        for w in range(CH // WT):  # 32 windows
            pg = ppB.tile([A, CH], f32, tag="pB")
            nc.tensor.matmul(
                pg[:, :], bd_s[:, :],
                xtile_r[:, w * CH : (w + 1) * CH], start=True, stop=True,
            )
            nc.vector.transpose(
                xa3[:, w * WT : (w + 1) * WT, :],
                pg.rearrange("p (t d) -> p t d", d=D)[:, :, :],
            )

        xa_r = bigp.tile([A, N * CH], f32r, tag="big")
        for s in range(TS):
            sl = slice(s * (N * CH // TS), (s + 1) * (N * CH // TS))
            nc.sync.dma_start(xa_r[:, sl], xa.bitcast(f32r)[:, sl])

        # ---------------- f phase: one node i per iteration ----------------
        for i in range(N):
            rhs = xa_r[:, i * CH : (i + 1) * CH]
            hf1 = workp.tile([A, BC], f32r, tag="w")
            tf = workp.tile([A, BC], f32r, tag="w")
            for h in range(2):
                pa = ppA.tile([A, 2 * CH], f32, tag="pA")
                for q in range(2):
                    c = h * 2 + q
                    nc.tensor.matmul(
                        pa[:, q * CH : (q + 1) * CH],
                        fw1p_s[:, c * A : (c + 1) * A], rhs,
                        start=True, stop=True,
                    )
                nc.scalar.activation(
                    hf1[:, h * 2 * CH : (h + 1) * 2 * CH], pa[:, :], LRELU,
                    bias=c2_s[:, i : i + 1], alpha=ALPHA,
                )
            for h in range(2):
                pa = ppA.tile([A, 2 * CH], f32, tag="pA")
                for q in range(2):
                    c = h * 2 + q
                    nc.tensor.matmul(
                        pa[:, q * CH : (q + 1) * CH], fw2_s[:, :],
                        hf1[:, c * CH : (c + 1) * CH], start=True, stop=True,
                    )
                nc.scalar.activation(
                    tf[:, h * 2 * CH : (h + 1) * 2 * CH], pa[:, :], LRELU,
                    bias=fb2_s[:, 0:1], alpha=ALPHA,
                )
            pr = ppR.tile([D, CH], f32, tag="pR")
            for c in range(NCH):
                base = (i + 1) * D - c
                lt = vp_s[:, base : base + D]
                nc.tensor.matmul(pr[:, :], lt, hf1[:, c * CH : (c + 1) * CH],
                                 start=(c == 0), stop=False)
                nc.tensor.matmul(pr[:, :], lt, tf[:, c * CH : (c + 1) * CH],
                                 start=False, stop=(c == NCH - 1))
            osb = outp.tile([NCH, CH], f32, tag="o")
            nc.vector.tensor_copy(osb[:, :], pr[:NCH, :])
            nc.sync.dma_start(
                out_d[i : i + 1, :].rearrange("o (c t) -> (o c) t", c=NCH),
                osb[:, :],
            )

    nc.compile()
    return nc


_NC_CACHE = {}


def _get_program(zero_b2=True):
    if zero_b2 not in _NC_CACHE:
        _NC_CACHE[zero_b2] = _build_program(zero_b2)
    return _NC_CACHE[zero_b2]


def _host_consts(W, embeddings, g_W1, g_b1, g_W2, g_b2, g_W3, g_b3,
                 f_W1, f_b1, f_W2, f_b2, f_W3, f_b3):
    f = np.float32
    W_adj = (W * (1.0 - np.eye(N, dtype=f))).astype(f)
    U = np.ascontiguousarray(g_W1[:D].T, dtype=f)                    # [A, N]
    C1 = np.ascontiguousarray((embeddings @ g_W1[D:] + g_b1).T, f)   # [A, N]
    s = W_adj.sum(axis=0)                                            # [N]
    C2 = (embeddings @ f_W1[D:] + f_b1 + np.outer(s, g_b3 @ f_W1[:D]))
    C2 = np.ascontiguousarray(C2.T, dtype=f)                         # [A, N]
    GW3P = np.zeros((A, NCH * A), f)
    FW1P = np.zeros((A, NCH * A), f)
    for c in range(NCH):
        GW3P[:, c * A + c * D : c * A + (c + 1) * D] = g_W3
        FW1P[c * D : (c + 1) * D, c * A : (c + 1) * A] = f_W1[:D]
    BD = np.kron(np.eye(NCH, dtype=f), W_adj).astype(f)
    VP = np.zeros((A, (N + 1) * D), f)
    for i in range(N):
        VP[:, (i + 1) * D] = f_W3[:, i]
    return {
        "GW2": np.ascontiguousarray(g_W2, f),
        "FW2": np.ascontiguousarray(f_W2, f),
        "GW3P": GW3P, "FW1P": FW1P, "BD": BD,
        "U": U, "C1": C1, "C2": C2,
        "GB2": np.ascontiguousarray(g_b2.reshape(A, 1), f),
        "FB2": np.ascontiguousarray(f_b2.reshape(A, 1), f),
        "VP": VP,
    }


def _kernel_numpy(X, W, embeddings, g_W1, g_b1, g_W2, g_b2, g_W3, g_b3,
                  f_W1, f_b1, f_W2, f_b2, f_W3, f_b3, group_mask):
    # general fallback (non-identity group_mask)
    def lrelu(x):
        return np.where(x > 0, x, ALPHA * x)

    def mlp(x, W1, b1, W2, b2, W3, b3):
        h = lrelu(x @ W1 + b1)
        h = h + lrelu(h @ W2 + b2)
        return h @ W3 + b3

    n = W.shape[0]
    W_adj = W * (1.0 - np.eye(n, dtype=W.dtype))
    Xm = X[:, None, :] * group_mask
    E = np.broadcast_to(embeddings, (X.shape[0], n, embeddings.shape[1]))
    Xe = mlp(np.concatenate([Xm, E], 2), g_W1, g_b1, g_W2, g_b2, g_W3, g_b3)
    Xa = np.einsum("ji,bjd->bid", W_adj, Xe)
    Xr = mlp(np.concatenate([Xa, E], 2), f_W1, f_b1, f_W2, f_b2, f_W3, f_b3)
    return (Xr * group_mask).sum(axis=1).astype(np.float32)


def kernel(X, W, embeddings, g_W1, g_b1, g_W2, g_b2, g_W3, g_b3,
           f_W1, f_b1, f_W2, f_b2, f_W3, f_b3, group_mask, _run_kw=None):
    if not np.allclose(group_mask, np.eye(N, D, dtype=np.float32)):
        return _kernel_numpy(X, W, embeddings, g_W1, g_b1, g_W2, g_b2, g_W3,
                             g_b3, f_W1, f_b1, f_W2, f_b2, f_W3, f_b3,
                             group_mask)

    from concourse import bass_utils

    consts = _host_consts(W, embeddings, g_W1, g_b1, g_W2, g_b2, g_W3, g_b3,
                          f_W1, f_b1, f_W2, f_b2, f_W3, f_b3)
    XT = np.ascontiguousarray(np.asarray(X, np.float32).T)  # [N, B]
    in_maps = []
    for k in range(NCORES):
        m = dict(consts)
        m["XT"] = np.ascontiguousarray(XT[:, k * BC : (k + 1) * BC])
        in_maps.append(m)

    nc = _get_program()
    res = bass_utils.run_bass_kernel_spmd(
        nc, in_maps, core_ids=list(range(NCORES)), **(_run_kw or {})
    )
    out = np.empty((B, D), np.float32)
    for k in range(NCORES):
        out[k * BC : (k + 1) * BC, :] = res.results[k]["OUT"].T
    out += f_b3.reshape(1, D).astype(np.float32)
    if _run_kw:
        kernel.last_results = res
    return out

